# revision 1
# baseline (speedup 1.0000x reference)
"""Depthwise 1d (per-channel linear) Trainium2 Bass kernel.

out[n, c, o] = sum_i x[n, c, i] * W[c, o, i] + b[c, o]
  x: [4096, 256, 64] f32, W: [256, 128, 64] f32, b: [256, 128] f32
  out: [4096, 256, 128] f32

Strategy: shard channels across 8 cores (32 channels/core, all 4096 rows).
Per-core weights are a single 1 MB block-diagonal fp16 tile loaded once,
so steady state moves only x in (33.5 MB) and out (67 MB) -- the kernel
is HBM-bound at ~358 GB/s/core.

Per n-tile of 128 rows: x loads naturally as [n, (c,i)]; PE-transposes
of 128x128 chunks (2 channels each) give lhsT = [(2ch x 64i), n]; each
channel pair is one fp16 matmul against a block-diagonal rhs [128, 256]
(upper-left = W_c0.T, lower-right = W_c1.T), keeping the full 128-row
contraction busy.  fp16 is ample precision for the 2e-2 gate (~1e-4).
The fp32->fp16 cast is fused into the PSUM evacuation of the transposes
on the ScalarE; bias is added on the DVE during PSUM evacuation of the
matmul results, against a partition-broadcast bias tile built once at
startup.  The PE stream is software-pipelined one iteration deep so
transposes for tile k+1 interleave ahead of tile k's matmuls.  Output
stores ride the ACT HWDGE ring so x loads never queue behind them.
"""

import os

# recover cleanly if a previous run left the NeuronCores wedged; must be
# set before the runtime initializes
os.environ.setdefault("NEURON_RT_RESET_CORES", "1")

import numpy as np

import concourse.bass as bass
import concourse.tile as tile
from concourse import bacc, mybir
from concourse.bass_utils import run_bass_kernel_spmd

N_CORES = 8
N, C, HI, HO = 4096, 256, 64, 128
CLOC = C // N_CORES  # 32 channels per core
NT = 128             # batch rows per tile

F32 = mybir.dt.float32
F16 = mybir.dt.float16


def build(n=N, cloc=CLOC, n_cores=N_CORES):
    nc = bacc.Bacc(
        "TRN2", target_bir_lowering=False, debug=False, num_devices=n_cores
    )
    pairs = cloc // 2
    x_d = nc.dram_tensor("x", [n, cloc, HI], F32, kind="ExternalInput").ap()
    # block-diagonal fp16 weights, host-packed: row k=(h*64+i), col block h
    w_d = nc.dram_tensor("wbd", [128, pairs, 2 * HO], F16, kind="ExternalInput").ap()
    b_d = nc.dram_tensor("bias", [cloc, HO], F32, kind="ExternalInput").ap()
    i_d = nc.dram_tensor("ident", [128, 128], F32, kind="ExternalInput").ap()
    o_d = nc.dram_tensor("out", [n, cloc, HO], F32, kind="ExternalOutput").ap()

    n_tiles = n // NT

    with tile.TileContext(nc) as tc:
        with (
            tc.tile_pool(name="const", bufs=1) as const,
            tc.tile_pool(name="xp", bufs=6) as xp,
            tc.tile_pool(name="xhp", bufs=4) as xhp,
            tc.tile_pool(name="op", bufs=4) as op,
            tc.tile_pool(name="pst", bufs=3, space="PSUM") as pst,
            tc.tile_pool(name="pso", bufs=5, space="PSUM") as pso,
        ):
            # first x tile loads go out before the constants so the DMA
            # engines ramp on the bulk stream immediately
            x_pre = []
            for ni in range(2):
                x_sb = xp.tile([128, cloc, HI], F32, name=f"x{ni}", tag="x")
                nc.sync.dma_start(out=x_sb, in_=x_d[ni * NT : (ni + 1) * NT, :, :])
                x_pre.append(x_sb)

            ident = const.tile([128, 128], F32, tag="ident")
            nc.sync.dma_start(out=ident, in_=i_d)
            wt = const.tile([128, pairs, 2 * HO], F16, tag="wt")
            nc.sync.dma_start(out=wt, in_=w_d)
            b_one = const.tile([1, cloc, HO], F32, tag="b_one")
            nc.sync.dma_start(out=b_one, in_=b_d)
            bias_sb = const.tile([128, cloc, HO], F32, tag="bias_sb")
            nc.gpsimd.partition_broadcast(bias_sb, b_one)

            def emit_T(ni):
                # x load, fp32 transposes, fused fp16 cast on PSUM
                # evacuation (ACT)
                n0 = ni * NT
                if ni < 2:
                    x_sb = x_pre[ni]
                else:
                    x_sb = xp.tile([128, cloc, HI], F32, name=f"x{ni}", tag="x")
                    nc.sync.dma_start(out=x_sb, in_=x_d[n0 : n0 + NT, :, :])
                xh_sb = xhp.tile([128, pairs, NT], F16, name=f"xh{ni}", tag="xh")
                for g in range(pairs // 4):  # 4 fp32 pairs per PSUM bank
                    ps = pst.tile([128, 4, NT], F32)
                    for q in range(4):
                        j = g * 4 + q
                        nc.tensor.transpose(
                            ps[:, q, :], x_sb[:, 2 * j : 2 * j + 2, :], ident
                        )
                    sl = slice(g * 4, (g + 1) * 4)
                    nc.scalar.copy(out=xh_sb[:, sl, :], in_=ps)
                return xh_sb

            def emit_M(ni, xh_sb):
                n0 = ni * NT
                o_sb = op.tile([128, cloc, HO], F32)
                half = cloc // 8  # matmul groups per half-tile store
                for g in range(cloc // 4):  # 4 channels / 2 pairs per bank
                    po = pso.tile([128, 4, HO], F32)
                    for p in range(2):
                        j = g * 2 + p
                        nc.tensor.matmul(
                            po[:, 2 * p : 2 * p + 2, :],
                            lhsT=xh_sb[:, j, :],
                            rhs=wt[:, j, :],
                            start=True,
                            stop=True,
                        )
                    nc.vector.tensor_add(
                        out=o_sb[:, g * 4 : (g + 1) * 4, :],
                        in0=po,
                        in1=bias_sb[:, g * 4 : (g + 1) * 4, :],
                    )
                    if (g + 1) % half == 0:
                        # store each half as soon as its adds land, so the
                        # store stream feeds the DMA engines smoothly
                        c0 = (g + 1 - half) * 4
                        c1 = (g + 1) * 4
                        nc.scalar.dma_start(
                            out=o_d[n0 : n0 + NT, c0:c1, :],
                            in_=o_sb[:, c0:c1, :],
                        )

            staged = emit_T(0)
            for ni in range(n_tiles):
                cur = staged
                # pipeline: next tile's transposes go to the PE ahead of
                # this tile's matmuls
                if ni + 1 < n_tiles:
                    staged = emit_T(ni + 1)
                emit_M(ni, cur)
    nc.compile()
    return nc


def pack_w(W):
    """[C, HO, HI] -> per-core block-diagonal [8, 128, C//16, 256] fp16."""
    C_, HO_, HI_ = W.shape
    pairs = C_ // (2 * N_CORES)
    Wt = W.astype(np.float16).transpose(0, 2, 1)  # [C, HI, HO] = W_c.T
    Wr = Wt.reshape(N_CORES, pairs, 2, HI_, HO_)
    out = np.zeros((N_CORES, 2, HI_, pairs, 2, HO_), dtype=np.float16)
    out[:, 0, :, :, 0, :] = Wr[:, :, 0].transpose(0, 2, 1, 3)
    out[:, 1, :, :, 1, :] = Wr[:, :, 1].transpose(0, 2, 1, 3)
    return np.ascontiguousarray(out.reshape(N_CORES, 128, pairs, 2 * HO_))


def make_in_maps(x, W, b):
    xs = np.asarray(x, dtype=np.float32)
    Wbd = pack_w(np.asarray(W, dtype=np.float32))
    bb = np.asarray(b, dtype=np.float32)
    ident = np.eye(128, dtype=np.float32)
    return [
        {
            "x": np.ascontiguousarray(xs[:, i * CLOC : (i + 1) * CLOC]),
            "wbd": Wbd[i],
            "bias": np.ascontiguousarray(bb[i * CLOC : (i + 1) * CLOC]),
            "ident": ident,
        }
        for i in range(N_CORES)
    ]


def assemble_out(results):
    return np.concatenate(
        [results[i]["out"] for i in range(N_CORES)], axis=1
    )


_cache = {}


def kernel(x, W, b):
    nc = _cache.get("nc")
    if nc is None:
        nc = _cache["nc"] = build()
    in_maps = make_in_maps(x, W, b)
    res = run_bass_kernel_spmd(nc, in_maps, core_ids=list(range(N_CORES)))
    return assemble_out(res.results)



# revision 7
# speedup vs baseline: 1.5373x; 1.5373x over previous
"""Depthwise 1d (per-channel linear) Trainium2 Bass kernel.

out[n, c, o] = sum_i x[n, c, i] * W[c, o, i] + b[c, o]
  x: [4096, 256, 64] f32, W: [256, 128, 64] f32, b: [256, 128] f32
  out: [4096, 256, 128] f32

Strategy: shard channels across 8 cores (32 channels/core, all 4096 rows).
The kernel is HBM-bound, so both streams move as fp16 (ample for the
2e-2 gate): x is cast + pre-transposed on the host to [pair, (2ch x 64i),
n] so the device does no transposes at all, and the output leaves the
device as fp16 [c, o, n] which the host casts/transposes back to f32.
Steady state moves only 16.8 MB in + 33.6 MB out per core, half the
fp32 traffic.

Per channel: out.T[o, n] = (W_c.T).T @ (x_c.T) -- the tiny W_c.T [64, 128]
is the PE-stationary operand (loaded once per 8 chunk-matmuls) and x
streams as the moving operand in 512-column PSUM chunks.  Evacuation
fuses the bias add: PSUM fp32 -> SBUF fp16 with the per-partition bias
column b[c, :] applied on ScalarE (Identity+bias) for half the chunks
and VectorE (tensor_scalar_add) for the other half, so no separate
bias pass exists.  x loads ride the sync HWDGE ring, output stores the
ACT ring; all DMAs move 1 MB blocks with 8 KB contiguous rows.
"""

import os

# recover cleanly if a previous run left the NeuronCores wedged; must be
# set before the runtime initializes
os.environ.setdefault("NEURON_RT_RESET_CORES", "1")

import numpy as np

import concourse.bass as bass
import concourse.tile as tile
from concourse import bacc, mybir
from concourse.bass_utils import run_bass_kernel_spmd

N_CORES = 8
N, C, HI, HO = 4096, 256, 64, 128
CLOC = C // N_CORES   # 32 channels per core
PAIRS = CLOC // 2     # 16 x-tiles of 128 partitions (2 channels each)
NCH = 512             # n-chunk per matmul == one PSUM bank of fp32
NCHUNKS = N // NCH    # 8

F32 = mybir.dt.float32
F16 = mybir.dt.float16


def build(n_cores=N_CORES):
    nc = bacc.Bacc(
        "TRN2", target_bir_lowering=False, debug=False, num_devices=n_cores
    )
    # x.T per pair: partition p = (channel 2j+p//64, feature p%64), free = n
    x_d = nc.dram_tensor("x", [PAIRS, 128, N], F16, kind="ExternalInput").ap()
    # W.T duplicated across both partition halves (PE needs lhsT and rhs
    # at the same base partition): wt[64*h + i, c, o] = W[c, o, i]
    w_d = nc.dram_tensor("wt", [128, CLOC, HO], F16, kind="ExternalInput").ap()
    # b.T: bt[o, c] = b[c, o]
    b_d = nc.dram_tensor("bt", [HO, CLOC], F32, kind="ExternalInput").ap()
    # transposed output: out[c, o, n]
    o_d = nc.dram_tensor("out", [CLOC, HO, N], F16, kind="ExternalOutput").ap()

    with tile.TileContext(nc) as tc:
        with (
            tc.tile_pool(name="const", bufs=1) as const,
            tc.tile_pool(name="xp", bufs=3) as xp,
            tc.tile_pool(name="op", bufs=4) as op,
            tc.tile_pool(name="psp", bufs=8, space="PSUM") as psp,
        ):
            def load_pair(p):
                t = xp.tile([128, N], F16, name=f"x{p}", tag="x")
                nc.sync.dma_start(out=t, in_=x_d[p])
                return t

            # first x tiles go out before the constants so the DMA
            # engines ramp on the bulk stream immediately
            x_tiles = [load_pair(0), load_pair(1)]

            wt = const.tile([128, CLOC, HO], F16, tag="wt")
            nc.sync.dma_start(out=wt, in_=w_d)
            bt = const.tile([HO, CLOC], F32, tag="bt")
            nc.sync.dma_start(out=bt, in_=b_d)
            # trigger the one-time ACT Identity table load while the
            # first x DMAs are still in flight
            warm = const.tile([HO, 1], F32, tag="warm")
            nc.scalar.add(out=warm, in_=bt[:, 0:1], add=bt[:, 1:2])

            def emit_pair(p, x_sb):
                for ci in range(2):
                    c = 2 * p + ci
                    o_sb = op.tile([HO, N], F16, name=f"o{c}", tag="o")
                    for k in range(NCHUNKS):
                        ps = psp.tile([HO, NCH], F32)
                        nc.tensor.matmul(
                            ps,
                            lhsT=wt[64 * ci : 64 * ci + 64, c, :],
                            rhs=x_sb[64 * ci : 64 * ci + 64, k * NCH : (k + 1) * NCH],
                            start=True,
                            stop=True,
                        )
                        # fused bias + fp16 cast on PSUM evacuation,
                        # alternating engines to split the load
                        sl = o_sb[:, k * NCH : (k + 1) * NCH]
                        if (k + ci) % 2 == 0:
                            nc.scalar.add(out=sl, in_=ps, add=bt[:, c : c + 1])
                        else:
                            nc.vector.tensor_scalar_add(sl, ps, bt[:, c : c + 1])
                    nc.scalar.dma_start(out=o_d[c], in_=o_sb)

            for p in range(PAIRS):
                # keep the x stream two tiles ahead of compute
                if p + 2 < PAIRS:
                    x_tiles.append(load_pair(p + 2))
                emit_pair(p, x_tiles[p])
    nc.compile()
    return nc


def make_in_maps(x, W, b):
    xh = np.asarray(x, dtype=np.float32).astype(np.float16)
    # [n, core, pair, ci, i] -> [core, pair, (ci, i), n]
    xt = np.ascontiguousarray(
        xh.reshape(N, N_CORES, PAIRS, 2, HI).transpose(1, 2, 3, 4, 0)
    ).reshape(N_CORES, PAIRS, 128, N)
    Wh = np.asarray(W, dtype=np.float32).astype(np.float16)
    wt1 = Wh.reshape(N_CORES, CLOC, HO, HI).transpose(0, 3, 1, 2)  # [core, i, c, o]
    wts = np.ascontiguousarray(
        np.concatenate([wt1, wt1], axis=1)
    )  # [core, 2*64 i, c, o] (duplicated halves)
    bb = np.asarray(b, dtype=np.float32)
    bts = np.ascontiguousarray(
        bb.reshape(N_CORES, CLOC, HO).transpose(0, 2, 1)
    )  # [core, o, c]
    return [
        {"x": xt[i], "wt": wts[i], "bt": bts[i]}
        for i in range(N_CORES)
    ]


def assemble_out(results):
    final = np.empty((N, C, HO), dtype=np.float32)
    fv = final.transpose(1, 2, 0)  # [C, HO, N] view of final
    for i in range(N_CORES):
        fv[i * CLOC : (i + 1) * CLOC] = results[i]["out"]
    return final


_cache = {}


def kernel(x, W, b):
    nc = _cache.get("nc")
    if nc is None:
        nc = _cache["nc"] = build()
    in_maps = make_in_maps(x, W, b)
    res = run_bass_kernel_spmd(nc, in_maps, core_ids=list(range(N_CORES)))
    return assemble_out(res.results)
